# revision 31
# baseline (speedup 1.0000x reference)
"""Multi-head attention (B=4, S=2048, D=1024, H=16) on 8 Trainium2 cores.

Sharding: data-parallel over batch (4) x tensor-parallel over heads (2).
Core c handles batch c//2 and heads (c%2)*8 .. +8.  Each core computes a
partial output (its heads' contribution through the O-projection); the host
sums the two partials per batch and adds the output bias.

Schedule: the attention phase is Scalar-engine bound (the softmax EXPs run
only on ACT at ~1 elem/cycle).  The kernel therefore software-pipelines
everything else around a continuous exp stream: a small projection lead-in
(kT/qT/v for head-pair 0), then 16 chunks (head-pair x query-block), each
issuing [scores(t) | AV(t-1) | normalize(t-1) | projection slices for the
next head-pair].  Scores run as 64x128 row-tiled pairs (both heads
concurrently on PE tiles T0/T8); AV matmuls are batched per chunk so the
PE only switches tiling mode twice per chunk.  Softmax normalization
broadcasts the denominator row via a DRAM bounce first and computes the
reciprocal on 64 partitions (not 1).  All matmul operands are bf16 (fp8
fails the accuracy budget); PSUM accumulation stays f32.
"""

import numpy as np
from contextlib import ExitStack

import ml_dtypes
import concourse.bass as bass
import concourse.tile as tile
from concourse import bacc, mybir
from concourse.bass import ts
from concourse.bass_utils import run_bass_kernel_spmd

P = 128
S = 2048          # sequence length
D = 1024          # model dim
DOUT = 512        # per-core projection width (8 heads x 64)
DK = 64           # head dim
B = 4
N_CORES = 8
F32 = mybir.dt.float32
BF16 = mybir.dt.bfloat16
FP = mybir.ActivationFunctionType

NKC = D // P      # 8 contraction chunks over model dim
NM = DOUT // P    # 4 dout chunks (also head pairs)
NQ = S // 512     # 4 query chunks of 512
NK16 = S // P     # 16 key chunks of 128

_cached_nc = None


def _emit(ctx: ExitStack, tc: "tile.TileContext", io: dict):
    nc = tc.nc

    qt_r = io["qt"].ap().rearrange("(c p) s -> p c s", p=P)      # [128, 8, 2048]
    kt_r = io["kt"].ap().rearrange("(c p) s -> p c s", p=P)
    vt_r = io["vt"].ap().rearrange("(c p) s -> p c s", p=P)
    wqt_r = io["wqt"].ap().rearrange("(c p) m -> p c m", p=P)    # [128, 8, 512]
    wkt_r = io["wkt"].ap().rearrange("(c p) m -> p c m", p=P)
    wvt_r = io["wvt"].ap().rearrange("(c p) m -> p c m", p=P)
    wot_r = io["wot"].ap().rearrange("(c p) n -> p c n", p=P)    # [128, 4, 1024]
    bq_r = io["bq"].ap().rearrange("(c p) -> p c", p=P)          # [128, 4]
    bk_r = io["bk"].ap().rearrange("(c p) -> p c", p=P)
    bv_ap = io["bv"].ap()                                        # [512]
    out_r = io["out"].ap().rearrange("(sc p) n -> p sc n", p=P)  # [128, 16, 1024]

    persist = ctx.enter_context(tc.tile_pool(name="persist", bufs=1))
    weights = ctx.enter_context(tc.tile_pool(name="weights", bufs=1))
    streams = ctx.enter_context(tc.tile_pool(name="streams", bufs=3))
    etp = ctx.enter_context(tc.tile_pool(name="etp", bufs=2 * NK16))
    avsb = ctx.enter_context(tc.tile_pool(name="avsb", bufs=1))
    recipp = ctx.enter_context(tc.tile_pool(name="recipp", bufs=1))
    stagp = ctx.enter_context(tc.tile_pool(name="stagp", bufs=2))
    outp = ctx.enter_context(tc.tile_pool(name="outp", bufs=2))
    dramp = ctx.enter_context(tc.tile_pool(name="dramp", bufs=2, space="DRAM"))

    ps_st = ctx.enter_context(tc.tile_pool(name="ps_st", bufs=3, space="PSUM"))
    ps_av = ctx.enter_context(tc.tile_pool(name="ps_av", bufs=2, space="PSUM"))

    # ---- constants / biases -------------------------------------------------
    bq_sb = persist.tile([P, NM], F32, tag="bq")
    nc.sync.dma_start(out=bq_sb, in_=bq_r)
    bk_sb = persist.tile([P, NM], F32, tag="bk")
    nc.sync.dma_start(out=bk_sb, in_=bk_r)
    # bv replicated across partitions (DMA partition-broadcast, stride 0)
    bv_rep = persist.tile([P, DOUT], F32, tag="bvrep")
    bv_bcast = bass.AP(
        tensor=bv_ap.tensor, offset=bv_ap.offset, ap=[[0, P]] + list(bv_ap.ap)
    )
    nc.gpsimd.dma_start(out=bv_rep, in_=bv_bcast)

    # ---- persistent activations (bf16) --------------------------------------
    # qT / kT: [dout, s] as 4 chunk-tiles of [128, 2048] (chunk = head pair)
    qT = [persist.tile([P, S], BF16, tag=f"qT{m}", name=f"qT{m}") for m in range(NM)]
    kT = [persist.tile([P, S], BF16, tag=f"kT{m}", name=f"kT{m}") for m in range(NM)]
    # v: [s, head, dk+1] tiles; col 64 of each head block holds ones so the
    # AV matmul's 65th output row accumulates the softmax denominator
    v_sb = [
        persist.tile([P, 8, 65], BF16, tag=f"v{i}", name=f"v{i}") for i in range(NK16)
    ]
    for i in range(NK16):
        nc.vector.memset(v_sb[i][:, :, 64:65], 1.0)
    # attn_outT: [dout, s] as 4 chunk-tiles (rows 0-63 even head, 64-127 odd)
    aoT = [persist.tile([P, S], BF16, tag=f"aoT{m}", name=f"aoT{m}") for m in range(NM)]

    # ---- weights (kept resident; DMAs staggered next to first use so the
    # lead-in xin streams aren't queued behind 4 MB of weights) -------------
    wk_sb = weights.tile([P, NKC, DOUT], BF16, tag="wk")
    nc.sync.dma_start(out=wk_sb, in_=wkt_r)
    wq_sb = weights.tile([P, NKC, DOUT], BF16, tag="wq")
    wv_sb = weights.tile([P, NKC, DOUT], BF16, tag="wv")
    wo_sb = weights.tile([P, NM, D], BF16, tag="wo")

    # ---- projection slice emitters ------------------------------------------
    def qk_slice(src_r, w_sb, bias_sb, dst, m, si):
        """One 512-query slice of the q or k projection for head-pair m."""
        xin = streams.tile([P, NKC, 512], BF16, tag="xin", name="xin")
        nc.sync.dma_start(out=xin, in_=src_r[:, :, ts(si, 512)])
        ps = ps_av.tile([P, 512], F32, tag="av", name="pj")
        for kc in range(NKC):
            nc.tensor.matmul(
                ps,
                lhsT=w_sb[:, kc, ts(m, P)],
                rhs=xin[:, kc, :],
                start=(kc == 0),
                stop=(kc == NKC - 1),
            )
        nc.vector.tensor_add(
            out=dst[m][:, ts(si, 512)],
            in0=ps,
            in1=bias_sb[:, m : m + 1].to_broadcast([P, 512]),
        )

    def v_slice(g):
        """v-projection (all 8 heads) over key block g (4 si16 chunks)."""
        vin = streams.tile([P, NKC, 512], BF16, tag="xin", name="vin")
        nc.sync.dma_start(out=vin, in_=vt_r[:, :, ts(g, 512)])
        for j in range(4):
            si16 = g * 4 + j
            ps = ps_av.tile([P, 512], F32, tag="av", name="pv")
            for kc in range(NKC):
                nc.tensor.matmul(
                    ps,
                    lhsT=vin[:, kc, ts(j, P)],
                    rhs=wv_sb[:, kc, :],
                    start=(kc == 0),
                    stop=(kc == NKC - 1),
                )
            nc.vector.tensor_add(
                out=v_sb[si16][:, :, 0:64],
                in0=ps.rearrange("p (h d) -> p h d", h=8),
                in1=bv_rep.rearrange("p (h d) -> p h d", h=8),
            )

    # ---- attention blocks ----------------------------------------------------
    def scores_block(pc, qi):
        """16 row-tiled score pairs + exps; returns the et tiles."""
        ets = []
        for ki in range(NK16):
            st = ps_st.tile([P, 1024], F32, tag="st", name="st")
            nc.tensor.matmul(
                st[:, 0:512],
                lhsT=kT[pc][0:64, ts(ki, P)],
                rhs=qT[pc][0:64, ts(qi, 512)],
                start=True,
                stop=True,
            )
            nc.tensor.matmul(
                st[:, 512:1024],
                lhsT=kT[pc][64:128, ts(ki, P)],
                rhs=qT[pc][64:128, ts(qi, 512)],
                start=True,
                stop=True,
                skip_group_check=True,
            )
            et = etp.tile([P, 1024], BF16, tag="et", name="et")
            nc.scalar.activation(out=et, in_=st, func=FP.Exp, scale=0.125)
            ets.append(et)
        return ets

    def av_block(pc, qi, ets):
        hh = 2 * pc
        av_e = ps_av.tile([P, 512], F32, tag="av", name="av_e")
        av_o = ps_av.tile([P, 512], F32, tag="av", name="av_o")
        for ki in range(NK16):
            first = ki == 0
            last = ki == NK16 - 1
            nc.tensor.matmul(
                av_e[0:65],
                lhsT=v_sb[ki][:, hh, :],
                rhs=ets[ki][:, 0:512],
                start=first,
                stop=last,
                skip_group_check=True,
            )
            nc.tensor.matmul(
                av_o[0:65],
                lhsT=v_sb[ki][:, hh + 1, :],
                rhs=ets[ki][:, 512:1024],
                start=first,
                stop=last,
                skip_group_check=True,
            )
        return [av_e, av_o]

    def epilogue(pc, qi, av):
        """Sum the split-K AV halves and softmax-normalize into aoT[pc]."""
        ae = avsb.tile([P, 1024], F32, tag="ae", name="ae")
        nc.vector.tensor_copy(out=ae[0:65, 0:512], in_=av[0][0:65])
        nc.vector.tensor_copy(out=ae[0:65, 512:1024], in_=av[1][0:65])
        # broadcast the denominator row to 64 partitions via a DRAM bounce,
        # THEN take the reciprocal (64 partitions wide, not 1)
        scr = dramp.tile([1, 1024], F32, tag="scr", name="scr")
        nc.sync.dma_start(out=scr, in_=ae[64:65, :])
        rep = recipp.tile([P, 1024], F32, tag="rep", name="rep")
        s0 = scr[0:1, :]
        nc.sync.dma_start(
            out=rep[0:64, :],
            in_=bass.AP(
                tensor=s0.tensor, offset=s0.offset, ap=[[0, 64]] + list(s0.ap[1:])
            ),
        )
        rrec = recipp.tile([P, 1024], F32, tag="rrec", name="rrec")
        nc.vector.reciprocal_approx_fast(out=rrec[0:64, :], in_=rep[0:64, :])
        nc.vector.tensor_mul(
            out=aoT[pc][0:64, ts(qi, 512)],
            in0=ae[0:64, 0:512],
            in1=rrec[0:64, 0:512],
        )
        stag = stagp.tile([P, 512], BF16, tag="stag", name="stag")
        nc.vector.tensor_mul(
            out=stag[0:64, :], in0=ae[0:64, 512:1024], in1=rrec[0:64, 512:1024]
        )
        nc.sync.dma_start(out=aoT[pc][64:128, ts(qi, 512)], in_=stag[0:64, :])

    def oproj(qi):
        """O-projection (partial) for the 4 si16 chunks of query block qi."""
        for si16 in range(4 * qi, 4 * qi + 4):
            for n2 in range(2):
                ps = ps_av.tile([P, 512], F32, tag="av", name="po")
                for c in range(NM):
                    nc.tensor.matmul(
                        ps,
                        lhsT=aoT[c][:, ts(si16, P)],
                        rhs=wo_sb[:, c, ts(n2, 512)],
                        start=(c == 0),
                        stop=(c == NM - 1),
                    )
                osb = outp.tile([P, 512], F32, tag="osb", name="osb")
                nc.vector.tensor_copy(out=osb, in_=ps)
                nc.sync.dma_start(out=out_r[:, si16, ts(n2, 512)], in_=osb)

    # ---- token dispatch ------------------------------------------------------
    def run_token(tok):
        kind = tok[0]
        if kind == "q":
            _, pc, si = tok
            qk_slice(qt_r, wq_sb, bq_sb, qT, pc, si)
        elif kind == "k":
            _, pc, si = tok
            qk_slice(kt_r, wk_sb, bk_sb, kT, pc, si)
        elif kind == "v":
            v_slice(tok[1])
        elif kind == "op":
            oproj(tok[1])

    # lead-in: everything scores(chunk 0) needs.  The kernel is PE-bound
    # (matmul issue overhead + weight loads put PE work above the scalar
    # exp chain), so the schedule optimizes PE density: V projection runs
    # full-width early, q/k slices spread so PE never starves late.
    run_token(("k", 0, 0))
    nc.sync.dma_start(out=wq_sb, in_=wqt_r)
    run_token(("q", 0, 0))
    nc.sync.dma_start(out=wv_sb, in_=wvt_r)
    for tok in [("k", 0, 1), ("k", 0, 2), ("k", 0, 3), ("v", 0), ("v", 1)]:
        run_token(tok)

    # Chunks run qi-major (pc inner): each query block's O-projection
    # becomes ready every 4 chunks instead of only after the last head
    # pair, so the op work spreads instead of piling into the tail.
    # pre_av[t]: v slices that must complete before AV(t-1)
    pre_av = {
        1: [("v", 3)],
    }
    # post[t]: projection / output work issued after AV(t-1)+epilogue(t-1)
    post = {
        0: [("k", 1, 0), ("k", 1, 1), ("k", 1, 2), ("k", 1, 3), ("q", 1, 0), ("v", 2)],
        1: ["WO", ("k", 2, 0), ("k", 2, 1), ("k", 2, 2), ("k", 2, 3), ("q", 2, 0)],
        2: [("k", 3, 0), ("k", 3, 1), ("k", 3, 2), ("k", 3, 3), ("q", 3, 0)],
        3: [("q", 0, 1)],
        4: [("op", 0), ("q", 1, 1)],
        5: [("q", 2, 1)],
        6: [("q", 3, 1)],
        7: [("q", 0, 2)],
        8: [("op", 1), ("q", 1, 2)],
        9: [("q", 2, 2)],
        10: [("q", 3, 2)],
        11: [("q", 0, 3)],
        12: [("op", 2), ("q", 1, 3)],
        13: [("q", 2, 3)],
        14: [("q", 3, 3)],
        15: [],
        16: [("op", 3)],
    }

    def flush_av(prev):
        ppc, pqi, pets = prev
        av = av_block(ppc, pqi, pets)
        epilogue(ppc, pqi, av)

    prev = None  # (pc, qi, ets) awaiting AV+epilogue
    for t in range(17):
        cur = None
        if t < 16:
            pc, qi = t % 4, t // 4
            cur = (pc, qi, scores_block(pc, qi))
        for tok in pre_av.get(t, []):
            run_token(tok)
        if prev is not None:
            flush_av(prev)
            prev = None
        prev = cur
        for tok in post.get(t, []):
            if tok == "AV_LAST":
                flush_av(prev)
                prev = None
            elif tok == "WV":
                nc.sync.dma_start(out=wv_sb, in_=wvt_r)
            elif tok == "WO":
                nc.sync.dma_start(out=wo_sb, in_=wot_r)
            else:
                run_token(tok)


def _build():
    global _cached_nc
    if _cached_nc is not None:
        return _cached_nc
    nc = bacc.Bacc("TRN2", target_bir_lowering=False, debug=False)
    io = {
        "qt": nc.dram_tensor("qt", [D, S], BF16, kind="ExternalInput"),
        "kt": nc.dram_tensor("kt", [D, S], BF16, kind="ExternalInput"),
        "vt": nc.dram_tensor("vt", [D, S], BF16, kind="ExternalInput"),
        "wqt": nc.dram_tensor("wqt", [D, DOUT], BF16, kind="ExternalInput"),
        "wkt": nc.dram_tensor("wkt", [D, DOUT], BF16, kind="ExternalInput"),
        "wvt": nc.dram_tensor("wvt", [D, DOUT], BF16, kind="ExternalInput"),
        "wot": nc.dram_tensor("wot", [DOUT, D], BF16, kind="ExternalInput"),
        "bq": nc.dram_tensor("bq", [DOUT], F32, kind="ExternalInput"),
        "bk": nc.dram_tensor("bk", [DOUT], F32, kind="ExternalInput"),
        "bv": nc.dram_tensor("bv", [DOUT], F32, kind="ExternalInput"),
        "out": nc.dram_tensor("out", [S, D], F32, kind="ExternalOutput"),
    }
    with tile.TileContext(nc) as tc:
        with ExitStack() as ctx:
            _emit(ctx, tc, io)
    nc.compile()
    _cached_nc = nc
    return nc


def make_in_maps(Q, K, V, Wq, bq, Wk, bk, Wv, bv, Wo):
    bf = lambda a: np.ascontiguousarray(np.asarray(a, np.float32)).astype(
        ml_dtypes.bfloat16
    )
    f = lambda a: np.ascontiguousarray(a, dtype=np.float32)
    in_maps = []
    for c in range(N_CORES):
        b = c // 2
        lo = (c % 2) * DOUT
        sl = slice(lo, lo + DOUT)
        in_maps.append(
            {
                "qt": bf(np.asarray(Q, np.float32)[b].T),
                "kt": bf(np.asarray(K, np.float32)[b].T),
                "vt": bf(np.asarray(V, np.float32)[b].T),
                "wqt": bf(np.asarray(Wq, np.float32)[sl, :].T),
                "wkt": bf(np.asarray(Wk, np.float32)[sl, :].T),
                "wvt": bf(np.asarray(Wv, np.float32)[sl, :].T),
                "wot": bf(np.asarray(Wo, np.float32)[:, sl].T),
                "bq": f(bq[sl]),
                "bk": f(bk[sl]),
                "bv": f(bv[sl]),
            }
        )
    return in_maps


def gather_output(results, bo):
    out = np.empty((B, S, D), dtype=np.float32)
    bo = np.asarray(bo, dtype=np.float32)
    for b in range(B):
        out[b] = results[2 * b]["out"] + results[2 * b + 1]["out"] + bo
    return out


def _numpy_fallback(Q, K, V, mask, Wq, bq, Wk, bk, Wv, bv, Wo, bo):
    """Exact reference math in numpy (only used if mask isn't all-ones)."""
    H, dk = 16, 64
    out = np.empty((B, S, D), dtype=np.float32)
    for b in range(B):
        q = (Q[b] @ Wq.T + bq).reshape(S, H, dk).transpose(1, 0, 2)
        k = (K[b] @ Wk.T + bk).reshape(S, H, dk).transpose(1, 0, 2)
        v = (V[b] @ Wv.T + bv).reshape(S, H, dk).transpose(1, 0, 2)
        o = np.empty((H, S, dk), dtype=np.float32)
        for h in range(H):
            s = (q[h] @ k[h].T) / np.sqrt(np.float32(dk))
            s = np.where(mask[b] == 0, np.float32(-1.0e9), s)
            s = s - s.max(axis=-1, keepdims=True)
            e = np.exp(s)
            a = e / e.sum(axis=-1, keepdims=True)
            o[h] = a @ v[h]
        out[b] = o.transpose(1, 0, 2).reshape(S, H * dk) @ Wo.T + bo
    return out


def kernel(Q, K, V, mask, Wq, bq, Wk, bk, Wv, bv, Wo, bo):
    Q = np.asarray(Q, dtype=np.float32)
    K = np.asarray(K, dtype=np.float32)
    V = np.asarray(V, dtype=np.float32)
    Wq = np.asarray(Wq, dtype=np.float32)
    Wk = np.asarray(Wk, dtype=np.float32)
    Wv = np.asarray(Wv, dtype=np.float32)
    Wo = np.asarray(Wo, dtype=np.float32)
    bq = np.asarray(bq, dtype=np.float32)
    bk = np.asarray(bk, dtype=np.float32)
    bv = np.asarray(bv, dtype=np.float32)
    bo = np.asarray(bo, dtype=np.float32)
    mask_np = np.asarray(mask)

    if not np.all(mask_np != 0):
        return _numpy_fallback(Q, K, V, mask_np, Wq, bq, Wk, bk, Wv, bv, Wo, bo)

    nc = _build()
    in_maps = make_in_maps(Q, K, V, Wq, bq, Wk, bk, Wv, bv, Wo)
    res = run_bass_kernel_spmd(nc, in_maps, list(range(N_CORES))).results
    return gather_output(res, bo)


# revision 32
# speedup vs baseline: 1.1137x; 1.1137x over previous
"""Multi-head attention (B=4, S=2048, D=1024, H=16) on 8 Trainium2 cores.

Sharding: data-parallel over batch (4) x tensor-parallel over heads (2).
Core c handles batch c//2 and heads (c%2)*8 .. +8.  Each core computes a
partial output (its heads' contribution through the O-projection); the host
sums the two partials per batch and adds the output bias.

Schedule: the attention phase is Scalar-engine bound (the softmax EXPs run
only on ACT at ~1 elem/cycle).  The kernel therefore software-pipelines
everything else around a continuous exp stream: a small projection lead-in
(kT/qT/v for head-pair 0), then 16 chunks (head-pair x query-block), each
issuing [scores(t) | AV(t-1) | normalize(t-1) | projection slices for the
next head-pair].  Scores run as 64x128 row-tiled pairs (both heads
concurrently on PE tiles T0/T8); AV matmuls are batched per chunk so the
PE only switches tiling mode twice per chunk.  Softmax normalization
broadcasts the denominator row via a DRAM bounce first and computes the
reciprocal on 64 partitions (not 1).  All matmul operands are bf16 (fp8
fails the accuracy budget); PSUM accumulation stays f32.
"""

import numpy as np
from contextlib import ExitStack

import ml_dtypes
import concourse.bass as bass
import concourse.tile as tile
from concourse import bacc, mybir
from concourse.bass import ts
from concourse.bass_utils import run_bass_kernel_spmd

P = 128
S = 2048          # sequence length
D = 1024          # model dim
DOUT = 512        # per-core projection width (8 heads x 64)
DK = 64           # head dim
B = 4
N_CORES = 8
F32 = mybir.dt.float32
BF16 = mybir.dt.bfloat16
FP = mybir.ActivationFunctionType

NKC = D // P      # 8 contraction chunks over model dim
NM = DOUT // P    # 4 dout chunks (also head pairs)
NQ = S // 512     # 4 query chunks of 512
NK16 = S // P     # 16 key chunks of 128

_cached_nc = None


def _emit(ctx: ExitStack, tc: "tile.TileContext", io: dict):
    nc = tc.nc

    qt_r = io["qt"].ap().rearrange("(c p) s -> p c s", p=P)      # [128, 8, 2048]
    kt_r = io["kt"].ap().rearrange("(c p) s -> p c s", p=P)
    vt_r = io["vt"].ap().rearrange("(c p) s -> p c s", p=P)
    wqt_r = io["wqt"].ap().rearrange("(c p) m -> p c m", p=P)    # [128, 8, 512]
    wkt_r = io["wkt"].ap().rearrange("(c p) m -> p c m", p=P)
    wvt_r = io["wvt"].ap().rearrange("(c p) m -> p c m", p=P)
    wot_r = io["wot"].ap().rearrange("(c p) n -> p c n", p=P)    # [128, 4, 1024]
    bq_r = io["bq"].ap().rearrange("(c p) -> p c", p=P)          # [128, 4]
    bk_r = io["bk"].ap().rearrange("(c p) -> p c", p=P)
    bv_ap = io["bv"].ap()                                        # [512]
    out_r = io["out"].ap().rearrange("(sc p) n -> p sc n", p=P)  # [128, 16, 1024]

    persist = ctx.enter_context(tc.tile_pool(name="persist", bufs=1))
    weights = ctx.enter_context(tc.tile_pool(name="weights", bufs=1))
    streams = ctx.enter_context(tc.tile_pool(name="streams", bufs=3))
    etp = ctx.enter_context(tc.tile_pool(name="etp", bufs=2 * NK16))
    avsb = ctx.enter_context(tc.tile_pool(name="avsb", bufs=1))
    recipp = ctx.enter_context(tc.tile_pool(name="recipp", bufs=1))
    stagp = ctx.enter_context(tc.tile_pool(name="stagp", bufs=2))
    outp = ctx.enter_context(tc.tile_pool(name="outp", bufs=2))
    dramp = ctx.enter_context(tc.tile_pool(name="dramp", bufs=2, space="DRAM"))

    ps_st = ctx.enter_context(tc.tile_pool(name="ps_st", bufs=3, space="PSUM"))
    ps_av = ctx.enter_context(tc.tile_pool(name="ps_av", bufs=2, space="PSUM"))

    # ---- constants / biases -------------------------------------------------
    bq_sb = persist.tile([P, NM], F32, tag="bq")
    nc.sync.dma_start(out=bq_sb, in_=bq_r)
    bk_sb = persist.tile([P, NM], F32, tag="bk")
    nc.sync.dma_start(out=bk_sb, in_=bk_r)
    # bv replicated across partitions (DMA partition-broadcast, stride 0)
    bv_rep = persist.tile([P, DOUT], F32, tag="bvrep")
    bv_bcast = bass.AP(
        tensor=bv_ap.tensor, offset=bv_ap.offset, ap=[[0, P]] + list(bv_ap.ap)
    )
    nc.gpsimd.dma_start(out=bv_rep, in_=bv_bcast)

    # ---- persistent activations (bf16) --------------------------------------
    # qT / kT: [dout, s] as 4 chunk-tiles of [128, 2048] (chunk = head pair)
    qT = [persist.tile([P, S], BF16, tag=f"qT{m}", name=f"qT{m}") for m in range(NM)]
    kT = [persist.tile([P, S], BF16, tag=f"kT{m}", name=f"kT{m}") for m in range(NM)]
    # v: [s, head, dk+1] tiles; col 64 of each head block holds ones so the
    # AV matmul's 65th output row accumulates the softmax denominator
    v_sb = [
        persist.tile([P, 8, 65], BF16, tag=f"v{i}", name=f"v{i}") for i in range(NK16)
    ]
    for i in range(NK16):
        nc.vector.memset(v_sb[i][:, :, 64:65], 1.0)
    # attn_outT: [dout, s] as 4 chunk-tiles (rows 0-63 even head, 64-127 odd)
    aoT = [persist.tile([P, S], BF16, tag=f"aoT{m}", name=f"aoT{m}") for m in range(NM)]

    # ---- weights (kept resident; DMAs staggered next to first use so the
    # lead-in xin streams aren't queued behind 4 MB of weights) -------------
    wk_sb = weights.tile([P, NKC, DOUT], BF16, tag="wk")
    nc.sync.dma_start(out=wk_sb, in_=wkt_r)
    wq_sb = weights.tile([P, NKC, DOUT], BF16, tag="wq")
    wv_sb = weights.tile([P, NKC, DOUT], BF16, tag="wv")
    wo_sb = weights.tile([P, NM, D], BF16, tag="wo")

    # ---- projection slice emitters ------------------------------------------
    def qk_slice(src_r, w_sb, bias_sb, dst, m, si):
        """One 512-query slice of the q or k projection for head-pair m."""
        xin = streams.tile([P, NKC, 512], BF16, tag="xin", name="xin")
        nc.sync.dma_start(out=xin, in_=src_r[:, :, ts(si, 512)])
        ps = ps_av.tile([P, 512], F32, tag="av", name="pj")
        for kc in range(NKC):
            nc.tensor.matmul(
                ps,
                lhsT=w_sb[:, kc, ts(m, P)],
                rhs=xin[:, kc, :],
                start=(kc == 0),
                stop=(kc == NKC - 1),
            )
        nc.vector.tensor_add(
            out=dst[m][:, ts(si, 512)],
            in0=ps,
            in1=bias_sb[:, m : m + 1].to_broadcast([P, 512]),
        )

    def v_slice(g):
        """v-projection (all 8 heads) over key block g (4 si16 chunks)."""
        vin = streams.tile([P, NKC, 512], BF16, tag="xin", name="vin")
        nc.sync.dma_start(out=vin, in_=vt_r[:, :, ts(g, 512)])
        for j in range(4):
            si16 = g * 4 + j
            ps = ps_av.tile([P, 512], F32, tag="av", name="pv")
            for kc in range(NKC):
                nc.tensor.matmul(
                    ps,
                    lhsT=vin[:, kc, ts(j, P)],
                    rhs=wv_sb[:, kc, :],
                    start=(kc == 0),
                    stop=(kc == NKC - 1),
                )
            nc.vector.tensor_add(
                out=v_sb[si16][:, :, 0:64],
                in0=ps.rearrange("p (h d) -> p h d", h=8),
                in1=bv_rep.rearrange("p (h d) -> p h d", h=8),
            )

    # ---- attention blocks ----------------------------------------------------
    def scores_block(pc, qi):
        """16 row-tiled score pairs + exps; returns the et tiles."""
        ets = []
        for ki in range(NK16):
            st = ps_st.tile([P, 1024], F32, tag="st", name="st")
            nc.tensor.matmul(
                st[:, 0:512],
                lhsT=kT[pc][0:64, ts(ki, P)],
                rhs=qT[pc][0:64, ts(qi, 512)],
                start=True,
                stop=True,
            )
            nc.tensor.matmul(
                st[:, 512:1024],
                lhsT=kT[pc][64:128, ts(ki, P)],
                rhs=qT[pc][64:128, ts(qi, 512)],
                start=True,
                stop=True,
                skip_group_check=True,
            )
            et = etp.tile([P, 1024], BF16, tag="et", name="et")
            nc.scalar.activation(out=et, in_=st, func=FP.Exp, scale=0.125)
            ets.append(et)
        return ets

    def av_block(pc, qi, ets):
        hh = 2 * pc
        av_e = ps_av.tile([P, 512], F32, tag="av", name="av_e")
        av_o = ps_av.tile([P, 512], F32, tag="av", name="av_o")
        for ki in range(NK16):
            first = ki == 0
            last = ki == NK16 - 1
            nc.tensor.matmul(
                av_e[0:65],
                lhsT=v_sb[ki][:, hh, :],
                rhs=ets[ki][:, 0:512],
                start=first,
                stop=last,
                skip_group_check=True,
            )
            nc.tensor.matmul(
                av_o[0:65],
                lhsT=v_sb[ki][:, hh + 1, :],
                rhs=ets[ki][:, 512:1024],
                start=first,
                stop=last,
                skip_group_check=True,
            )
        return [av_e, av_o]

    def epilogue(pc, qi, av):
        """Sum the split-K AV halves and softmax-normalize into aoT[pc]."""
        ae = avsb.tile([P, 1024], F32, tag="ae", name="ae")
        nc.vector.tensor_copy(out=ae[0:65, 0:512], in_=av[0][0:65])
        nc.vector.tensor_copy(out=ae[0:65, 512:1024], in_=av[1][0:65])
        # broadcast the denominator row to 64 partitions via a DRAM bounce,
        # THEN take the reciprocal (64 partitions wide, not 1)
        scr = dramp.tile([1, 1024], F32, tag="scr", name="scr")
        nc.sync.dma_start(out=scr, in_=ae[64:65, :])
        rep = recipp.tile([P, 1024], F32, tag="rep", name="rep")
        s0 = scr[0:1, :]
        nc.sync.dma_start(
            out=rep[0:64, :],
            in_=bass.AP(
                tensor=s0.tensor, offset=s0.offset, ap=[[0, 64]] + list(s0.ap[1:])
            ),
        )
        rrec = recipp.tile([P, 1024], F32, tag="rrec", name="rrec")
        nc.vector.reciprocal_approx_fast(out=rrec[0:64, :], in_=rep[0:64, :])
        nc.vector.tensor_mul(
            out=aoT[pc][0:64, ts(qi, 512)],
            in0=ae[0:64, 0:512],
            in1=rrec[0:64, 0:512],
        )
        stag = stagp.tile([P, 512], BF16, tag="stag", name="stag")
        nc.vector.tensor_mul(
            out=stag[0:64, :], in0=ae[0:64, 512:1024], in1=rrec[0:64, 512:1024]
        )
        nc.sync.dma_start(out=aoT[pc][64:128, ts(qi, 512)], in_=stag[0:64, :])

    def oproj(qi):
        """O-projection (partial) for the 4 si16 chunks of query block qi."""
        for si16 in range(4 * qi, 4 * qi + 4):
            for n2 in range(2):
                ps = ps_av.tile([P, 512], F32, tag="av", name="po")
                for c in range(NM):
                    nc.tensor.matmul(
                        ps,
                        lhsT=aoT[c][:, ts(si16, P)],
                        rhs=wo_sb[:, c, ts(n2, 512)],
                        start=(c == 0),
                        stop=(c == NM - 1),
                    )
                osb = outp.tile([P, 512], F32, tag="osb", name="osb")
                nc.vector.tensor_copy(out=osb, in_=ps)
                nc.sync.dma_start(out=out_r[:, si16, ts(n2, 512)], in_=osb)

    # ---- token dispatch ------------------------------------------------------
    def run_token(tok):
        kind = tok[0]
        if kind == "q":
            _, pc, si = tok
            qk_slice(qt_r, wq_sb, bq_sb, qT, pc, si)
        elif kind == "k":
            _, pc, si = tok
            qk_slice(kt_r, wk_sb, bk_sb, kT, pc, si)
        elif kind == "v":
            v_slice(tok[1])
        elif kind == "op":
            oproj(tok[1])

    # lead-in: everything scores(chunk 0) needs.  The kernel is PE-bound
    # (matmul issue overhead + weight loads put PE work above the scalar
    # exp chain), so the schedule optimizes PE density: V projection runs
    # full-width early, q/k slices spread so PE never starves late.
    run_token(("k", 0, 0))
    nc.sync.dma_start(out=wq_sb, in_=wqt_r)
    run_token(("q", 0, 0))
    nc.sync.dma_start(out=wv_sb, in_=wvt_r)
    for tok in [("k", 0, 1), ("k", 0, 2), ("k", 0, 3)]:
        run_token(tok)

    # pre_av[t]: v slices that must complete before AV(t-1)
    pre_av = {
        1: [("v", 3)],
    }
    # post[t]: projection / output work issued after AV(t-1)+epilogue(t-1)
    post = {
        0: [("v", 0), ("v", 1), ("v", 2), ("q", 0, 1)],
        1: ["WO", ("q", 0, 2), ("q", 0, 3), ("k", 1, 0)],
        2: [("k", 1, 1), ("k", 1, 2), ("k", 1, 3)],
        3: [("q", 1, 0), ("q", 1, 1), ("q", 1, 2)],
        4: [("q", 1, 3), ("k", 2, 0), ("k", 2, 1)],
        5: [("k", 2, 2), ("k", 2, 3), ("q", 2, 0)],
        6: [("q", 2, 1), ("q", 2, 2), ("q", 2, 3)],
        7: [("k", 3, 0)],
        8: [("k", 3, 1), ("q", 3, 0)],
        9: [("k", 3, 2), ("q", 3, 1)],
        10: [("k", 3, 3), ("q", 3, 2)],
        11: [],
        12: [("q", 3, 3)],
        13: [("op", 0)],
        14: [("op", 1)],
        15: ["AV_LAST", ("op", 2)],
        16: [("op", 3)],
    }

    def flush_av(prev):
        ppc, pqi, pets = prev
        av = av_block(ppc, pqi, pets)
        epilogue(ppc, pqi, av)

    prev = None  # (pc, qi, ets) awaiting AV+epilogue
    for t in range(17):
        cur = None
        if t < 16:
            pc, qi = t // 4, t % 4
            cur = (pc, qi, scores_block(pc, qi))
        for tok in pre_av.get(t, []):
            run_token(tok)
        if prev is not None:
            flush_av(prev)
            prev = None
        prev = cur
        for tok in post.get(t, []):
            if tok == "AV_LAST":
                flush_av(prev)
                prev = None
            elif tok == "WV":
                nc.sync.dma_start(out=wv_sb, in_=wvt_r)
            elif tok == "WO":
                nc.sync.dma_start(out=wo_sb, in_=wot_r)
            else:
                run_token(tok)


def _build():
    global _cached_nc
    if _cached_nc is not None:
        return _cached_nc
    nc = bacc.Bacc("TRN2", target_bir_lowering=False, debug=False)
    io = {
        "qt": nc.dram_tensor("qt", [D, S], BF16, kind="ExternalInput"),
        "kt": nc.dram_tensor("kt", [D, S], BF16, kind="ExternalInput"),
        "vt": nc.dram_tensor("vt", [D, S], BF16, kind="ExternalInput"),
        "wqt": nc.dram_tensor("wqt", [D, DOUT], BF16, kind="ExternalInput"),
        "wkt": nc.dram_tensor("wkt", [D, DOUT], BF16, kind="ExternalInput"),
        "wvt": nc.dram_tensor("wvt", [D, DOUT], BF16, kind="ExternalInput"),
        "wot": nc.dram_tensor("wot", [DOUT, D], BF16, kind="ExternalInput"),
        "bq": nc.dram_tensor("bq", [DOUT], F32, kind="ExternalInput"),
        "bk": nc.dram_tensor("bk", [DOUT], F32, kind="ExternalInput"),
        "bv": nc.dram_tensor("bv", [DOUT], F32, kind="ExternalInput"),
        "out": nc.dram_tensor("out", [S, D], F32, kind="ExternalOutput"),
    }
    with tile.TileContext(nc) as tc:
        with ExitStack() as ctx:
            _emit(ctx, tc, io)
    nc.compile()
    _cached_nc = nc
    return nc


def make_in_maps(Q, K, V, Wq, bq, Wk, bk, Wv, bv, Wo):
    bf = lambda a: np.ascontiguousarray(np.asarray(a, np.float32)).astype(
        ml_dtypes.bfloat16
    )
    f = lambda a: np.ascontiguousarray(a, dtype=np.float32)
    in_maps = []
    for c in range(N_CORES):
        b = c // 2
        lo = (c % 2) * DOUT
        sl = slice(lo, lo + DOUT)
        in_maps.append(
            {
                "qt": bf(np.asarray(Q, np.float32)[b].T),
                "kt": bf(np.asarray(K, np.float32)[b].T),
                "vt": bf(np.asarray(V, np.float32)[b].T),
                "wqt": bf(np.asarray(Wq, np.float32)[sl, :].T),
                "wkt": bf(np.asarray(Wk, np.float32)[sl, :].T),
                "wvt": bf(np.asarray(Wv, np.float32)[sl, :].T),
                "wot": bf(np.asarray(Wo, np.float32)[:, sl].T),
                "bq": f(bq[sl]),
                "bk": f(bk[sl]),
                "bv": f(bv[sl]),
            }
        )
    return in_maps


def gather_output(results, bo):
    out = np.empty((B, S, D), dtype=np.float32)
    bo = np.asarray(bo, dtype=np.float32)
    for b in range(B):
        out[b] = results[2 * b]["out"] + results[2 * b + 1]["out"] + bo
    return out


def _numpy_fallback(Q, K, V, mask, Wq, bq, Wk, bk, Wv, bv, Wo, bo):
    """Exact reference math in numpy (only used if mask isn't all-ones)."""
    H, dk = 16, 64
    out = np.empty((B, S, D), dtype=np.float32)
    for b in range(B):
        q = (Q[b] @ Wq.T + bq).reshape(S, H, dk).transpose(1, 0, 2)
        k = (K[b] @ Wk.T + bk).reshape(S, H, dk).transpose(1, 0, 2)
        v = (V[b] @ Wv.T + bv).reshape(S, H, dk).transpose(1, 0, 2)
        o = np.empty((H, S, dk), dtype=np.float32)
        for h in range(H):
            s = (q[h] @ k[h].T) / np.sqrt(np.float32(dk))
            s = np.where(mask[b] == 0, np.float32(-1.0e9), s)
            s = s - s.max(axis=-1, keepdims=True)
            e = np.exp(s)
            a = e / e.sum(axis=-1, keepdims=True)
            o[h] = a @ v[h]
        out[b] = o.transpose(1, 0, 2).reshape(S, H * dk) @ Wo.T + bo
    return out


def kernel(Q, K, V, mask, Wq, bq, Wk, bk, Wv, bv, Wo, bo):
    Q = np.asarray(Q, dtype=np.float32)
    K = np.asarray(K, dtype=np.float32)
    V = np.asarray(V, dtype=np.float32)
    Wq = np.asarray(Wq, dtype=np.float32)
    Wk = np.asarray(Wk, dtype=np.float32)
    Wv = np.asarray(Wv, dtype=np.float32)
    Wo = np.asarray(Wo, dtype=np.float32)
    bq = np.asarray(bq, dtype=np.float32)
    bk = np.asarray(bk, dtype=np.float32)
    bv = np.asarray(bv, dtype=np.float32)
    bo = np.asarray(bo, dtype=np.float32)
    mask_np = np.asarray(mask)

    if not np.all(mask_np != 0):
        return _numpy_fallback(Q, K, V, mask_np, Wq, bq, Wk, bk, Wv, bv, Wo, bo)

    nc = _build()
    in_maps = make_in_maps(Q, K, V, Wq, bq, Wk, bk, Wv, bv, Wo)
    res = run_bass_kernel_spmd(nc, in_maps, list(range(N_CORES))).results
    return gather_output(res, bo)


# revision 35
# speedup vs baseline: 1.1226x; 1.0079x over previous
"""Multi-head attention (B=4, S=2048, D=1024, H=16) on 8 Trainium2 cores.

Sharding: data-parallel over batch (4) x tensor-parallel over heads (2).
Core c handles batch c//2 and heads (c%2)*8 .. +8.  Each core computes a
partial output (its heads' contribution through the O-projection); the host
sums the two partials per batch and adds the output bias.

Schedule: the attention phase is Scalar-engine bound (the softmax EXPs run
only on ACT at ~1 elem/cycle).  The kernel therefore software-pipelines
everything else around a continuous exp stream: a small projection lead-in
(kT/qT/v for head-pair 0), then 16 chunks (head-pair x query-block), each
issuing [scores(t) | AV(t-1) | normalize(t-1) | projection slices for the
next head-pair].  Scores run as 64x128 row-tiled pairs (both heads
concurrently on PE tiles T0/T8); AV matmuls are batched per chunk so the
PE only switches tiling mode twice per chunk.  Softmax normalization
broadcasts the denominator row via a DRAM bounce first and computes the
reciprocal on 64 partitions (not 1).  All matmul operands are bf16 (fp8
fails the accuracy budget); PSUM accumulation stays f32.
"""

import numpy as np
from contextlib import ExitStack

import ml_dtypes
import concourse.bass as bass
import concourse.tile as tile
from concourse import bacc, mybir
from concourse.bass import ts
from concourse.bass_utils import run_bass_kernel_spmd

P = 128
S = 2048          # sequence length
D = 1024          # model dim
DOUT = 512        # per-core projection width (8 heads x 64)
DK = 64           # head dim
B = 4
N_CORES = 8
F32 = mybir.dt.float32
BF16 = mybir.dt.bfloat16
FP = mybir.ActivationFunctionType

NKC = D // P      # 8 contraction chunks over model dim
NM = DOUT // P    # 4 dout chunks (also head pairs)
NQ = S // 512     # 4 query chunks of 512
NK16 = S // P     # 16 key chunks of 128

_cached_nc = None


def _emit(ctx: ExitStack, tc: "tile.TileContext", io: dict):
    nc = tc.nc

    qt_r = io["qt"].ap().rearrange("(c p) s -> p c s", p=P)      # [128, 8, 2048]
    kt_r = io["kt"].ap().rearrange("(c p) s -> p c s", p=P)
    vt_r = io["vt"].ap().rearrange("(c p) s -> p c s", p=P)
    wqt_r = io["wqt"].ap().rearrange("(c p) m -> p c m", p=P)    # [128, 8, 512]
    wkt_r = io["wkt"].ap().rearrange("(c p) m -> p c m", p=P)
    wvt_r = io["wvt"].ap().rearrange("(c p) m -> p c m", p=P)
    wot_r = io["wot"].ap().rearrange("(c p) n -> p c n", p=P)    # [128, 4, 1024]
    bq_r = io["bq"].ap().rearrange("(c p) -> p c", p=P)          # [128, 4]
    bk_r = io["bk"].ap().rearrange("(c p) -> p c", p=P)
    bv_ap = io["bv"].ap()                                        # [512]
    out_r = io["out"].ap().rearrange("(sc p) n -> p sc n", p=P)  # [128, 16, 1024]

    persist = ctx.enter_context(tc.tile_pool(name="persist", bufs=1))
    weights = ctx.enter_context(tc.tile_pool(name="weights", bufs=1))
    streams = ctx.enter_context(tc.tile_pool(name="streams", bufs=3))
    etp = ctx.enter_context(tc.tile_pool(name="etp", bufs=2 * NK16))
    avsb = ctx.enter_context(tc.tile_pool(name="avsb", bufs=1))
    recipp = ctx.enter_context(tc.tile_pool(name="recipp", bufs=1))
    stagp = ctx.enter_context(tc.tile_pool(name="stagp", bufs=2))
    outp = ctx.enter_context(tc.tile_pool(name="outp", bufs=2))
    dramp = ctx.enter_context(tc.tile_pool(name="dramp", bufs=2, space="DRAM"))

    ps_st = ctx.enter_context(tc.tile_pool(name="ps_st", bufs=3, space="PSUM"))
    ps_av = ctx.enter_context(tc.tile_pool(name="ps_av", bufs=2, space="PSUM"))

    # ---- constants / biases -------------------------------------------------
    bq_sb = persist.tile([P, NM], F32, tag="bq")
    nc.sync.dma_start(out=bq_sb, in_=bq_r)
    bk_sb = persist.tile([P, NM], F32, tag="bk")
    nc.sync.dma_start(out=bk_sb, in_=bk_r)
    # bv replicated across partitions (DMA partition-broadcast, stride 0)
    bv_rep = persist.tile([P, DOUT], F32, tag="bvrep")
    bv_bcast = bass.AP(
        tensor=bv_ap.tensor, offset=bv_ap.offset, ap=[[0, P]] + list(bv_ap.ap)
    )
    nc.gpsimd.dma_start(out=bv_rep, in_=bv_bcast)

    # ---- persistent activations (bf16) --------------------------------------
    # qT / kT: [dout, s] as 4 chunk-tiles of [128, 2048] (chunk = head pair)
    qT = [persist.tile([P, S], BF16, tag=f"qT{m}", name=f"qT{m}") for m in range(NM)]
    kT = [persist.tile([P, S], BF16, tag=f"kT{m}", name=f"kT{m}") for m in range(NM)]
    # v: [s, head, dk+1] tiles; col 64 of each head block holds ones so the
    # AV matmul's 65th output row accumulates the softmax denominator
    v_sb = [
        persist.tile([P, 8, 65], BF16, tag=f"v{i}", name=f"v{i}") for i in range(NK16)
    ]
    for i in range(NK16):
        nc.vector.memset(v_sb[i][:, :, 64:65], 1.0)
    # attn_outT: [dout, s] as 4 chunk-tiles (rows 0-63 even head, 64-127 odd)
    aoT = [persist.tile([P, S], BF16, tag=f"aoT{m}", name=f"aoT{m}") for m in range(NM)]

    # ---- weights (kept resident; DMAs staggered next to first use so the
    # lead-in xin streams aren't queued behind 4 MB of weights) -------------
    wk_sb = weights.tile([P, NKC, DOUT], BF16, tag="wk")
    for kq in range(4):
        nc.sync.dma_start(
            out=wk_sb[:, 2 * kq : 2 * kq + 2, :], in_=wkt_r[:, 2 * kq : 2 * kq + 2, :]
        )
    wq_sb = weights.tile([P, NKC, DOUT], BF16, tag="wq")
    wv_sb = weights.tile([P, NKC, DOUT], BF16, tag="wv")
    wo_sb = weights.tile([P, NM, D], BF16, tag="wo")

    # ---- projection slice emitters ------------------------------------------
    def qk_slice(src_r, w_sb, bias_sb, dst, m, si, split=False):
        """One 512-query slice of the q or k projection for head-pair m.

        split=True issues the xin DMA in quarters so the first matmul only
        waits for the first 256 KB (lead-in latency trim).
        """
        xin = streams.tile([P, NKC, 512], BF16, tag="xin", name="xin")
        if split:
            for kq in range(4):
                nc.sync.dma_start(
                    out=xin[:, 2 * kq : 2 * kq + 2, :],
                    in_=src_r[:, 2 * kq : 2 * kq + 2, ts(si, 512)],
                )
        else:
            nc.sync.dma_start(out=xin, in_=src_r[:, :, ts(si, 512)])
        ps = ps_av.tile([P, 512], F32, tag="av", name="pj")
        for kc in range(NKC):
            nc.tensor.matmul(
                ps,
                lhsT=w_sb[:, kc, ts(m, P)],
                rhs=xin[:, kc, :],
                start=(kc == 0),
                stop=(kc == NKC - 1),
            )
        nc.vector.tensor_add(
            out=dst[m][:, ts(si, 512)],
            in0=ps,
            in1=bias_sb[:, m : m + 1].to_broadcast([P, 512]),
        )

    def v_slice(g):
        """v-projection (all 8 heads) over key block g (4 si16 chunks)."""
        vin = streams.tile([P, NKC, 512], BF16, tag="xin", name="vin")
        nc.sync.dma_start(out=vin, in_=vt_r[:, :, ts(g, 512)])
        for j in range(4):
            si16 = g * 4 + j
            ps = ps_av.tile([P, 512], F32, tag="av", name="pv")
            for kc in range(NKC):
                nc.tensor.matmul(
                    ps,
                    lhsT=vin[:, kc, ts(j, P)],
                    rhs=wv_sb[:, kc, :],
                    start=(kc == 0),
                    stop=(kc == NKC - 1),
                )
            nc.vector.tensor_add(
                out=v_sb[si16][:, :, 0:64],
                in0=ps.rearrange("p (h d) -> p h d", h=8),
                in1=bv_rep.rearrange("p (h d) -> p h d", h=8),
            )

    # ---- attention blocks ----------------------------------------------------
    def scores_block(pc, qi):
        """16 row-tiled score pairs + exps; returns the et tiles."""
        ets = []
        for ki in range(NK16):
            st = ps_st.tile([P, 1024], F32, tag="st", name="st")
            nc.tensor.matmul(
                st[:, 0:512],
                lhsT=kT[pc][0:64, ts(ki, P)],
                rhs=qT[pc][0:64, ts(qi, 512)],
                start=True,
                stop=True,
            )
            nc.tensor.matmul(
                st[:, 512:1024],
                lhsT=kT[pc][64:128, ts(ki, P)],
                rhs=qT[pc][64:128, ts(qi, 512)],
                start=True,
                stop=True,
                skip_group_check=True,
            )
            et = etp.tile([P, 1024], BF16, tag="et", name="et")
            nc.scalar.activation(out=et, in_=st, func=FP.Exp, scale=0.125)
            ets.append(et)
        return ets

    def av_block(pc, qi, ets):
        hh = 2 * pc
        av_e = ps_av.tile([P, 512], F32, tag="av", name="av_e")
        av_o = ps_av.tile([P, 512], F32, tag="av", name="av_o")
        for ki in range(NK16):
            first = ki == 0
            last = ki == NK16 - 1
            nc.tensor.matmul(
                av_e[0:65],
                lhsT=v_sb[ki][:, hh, :],
                rhs=ets[ki][:, 0:512],
                start=first,
                stop=last,
                skip_group_check=True,
            )
            nc.tensor.matmul(
                av_o[0:65],
                lhsT=v_sb[ki][:, hh + 1, :],
                rhs=ets[ki][:, 512:1024],
                start=first,
                stop=last,
                skip_group_check=True,
            )
        return [av_e, av_o]

    def epilogue(pc, qi, av):
        """Sum the split-K AV halves and softmax-normalize into aoT[pc]."""
        ae = avsb.tile([P, 1024], F32, tag="ae", name="ae")
        nc.vector.tensor_copy(out=ae[0:65, 0:512], in_=av[0][0:65])
        nc.vector.tensor_copy(out=ae[0:65, 512:1024], in_=av[1][0:65])
        # broadcast the denominator row to 64 partitions via a DRAM bounce,
        # THEN take the reciprocal (64 partitions wide, not 1)
        scr = dramp.tile([1, 1024], F32, tag="scr", name="scr")
        nc.sync.dma_start(out=scr, in_=ae[64:65, :])
        rep = recipp.tile([P, 1024], F32, tag="rep", name="rep")
        s0 = scr[0:1, :]
        nc.sync.dma_start(
            out=rep[0:64, :],
            in_=bass.AP(
                tensor=s0.tensor, offset=s0.offset, ap=[[0, 64]] + list(s0.ap[1:])
            ),
        )
        rrec = recipp.tile([P, 1024], F32, tag="rrec", name="rrec")
        nc.vector.reciprocal_approx_fast(out=rrec[0:64, :], in_=rep[0:64, :])
        nc.vector.tensor_mul(
            out=aoT[pc][0:64, ts(qi, 512)],
            in0=ae[0:64, 0:512],
            in1=rrec[0:64, 0:512],
        )
        stag = stagp.tile([P, 512], BF16, tag="stag", name="stag")
        nc.vector.tensor_mul(
            out=stag[0:64, :], in0=ae[0:64, 512:1024], in1=rrec[0:64, 512:1024]
        )
        nc.sync.dma_start(out=aoT[pc][64:128, ts(qi, 512)], in_=stag[0:64, :])

    def oproj(qi):
        """O-projection (partial) for the 4 si16 chunks of query block qi."""
        for si16 in range(4 * qi, 4 * qi + 4):
            for n2 in range(2):
                ps = ps_av.tile([P, 512], F32, tag="av", name="po")
                for c in range(NM):
                    nc.tensor.matmul(
                        ps,
                        lhsT=aoT[c][:, ts(si16, P)],
                        rhs=wo_sb[:, c, ts(n2, 512)],
                        start=(c == 0),
                        stop=(c == NM - 1),
                    )
                osb = outp.tile([P, 512], F32, tag="osb", name="osb")
                nc.vector.tensor_copy(out=osb, in_=ps)
                nc.sync.dma_start(out=out_r[:, si16, ts(n2, 512)], in_=osb)

    # ---- token dispatch ------------------------------------------------------
    def run_token(tok):
        kind = tok[0]
        if kind == "q":
            _, pc, si = tok
            qk_slice(qt_r, wq_sb, bq_sb, qT, pc, si)
        elif kind == "k":
            _, pc, si = tok
            qk_slice(kt_r, wk_sb, bk_sb, kT, pc, si, split=(pc == 0 and si == 0))
        elif kind == "v":
            v_slice(tok[1])
        elif kind == "op":
            oproj(tok[1])

    # lead-in: everything scores(chunk 0) needs.  The kernel is PE-bound
    # (matmul issue overhead + weight loads put PE work above the scalar
    # exp chain), so the schedule optimizes PE density: V projection runs
    # full-width early, q/k slices spread so PE never starves late.
    run_token(("k", 0, 0))
    nc.sync.dma_start(out=wq_sb, in_=wqt_r)
    run_token(("q", 0, 0))
    nc.sync.dma_start(out=wv_sb, in_=wvt_r)
    for tok in [("k", 0, 1), ("k", 0, 2), ("k", 0, 3)]:
        run_token(tok)

    # pre_av[t]: v slices that must complete before AV(t-1)
    pre_av = {
        1: [("v", 3)],
    }
    # post[t]: projection / output work issued after AV(t-1)+epilogue(t-1)
    post = {
        0: [("v", 0), ("v", 1), ("v", 2), ("q", 0, 1)],
        1: ["WO", ("q", 0, 2), ("q", 0, 3), ("k", 1, 0)],
        2: [("k", 1, 1), ("k", 1, 2), ("k", 1, 3)],
        3: [("q", 1, 0), ("q", 1, 1), ("q", 1, 2)],
        4: [("q", 1, 3), ("k", 2, 0), ("k", 2, 1)],
        5: [("k", 2, 2), ("k", 2, 3), ("q", 2, 0)],
        6: [("q", 2, 1), ("q", 2, 2), ("q", 2, 3)],
        7: [("k", 3, 0)],
        8: [("k", 3, 1), ("q", 3, 0)],
        9: [("k", 3, 2), ("q", 3, 1)],
        10: [("k", 3, 3), ("q", 3, 2)],
        11: [],
        12: [("q", 3, 3)],
        13: [("op", 0)],
        14: [("op", 1)],
        15: ["AV_LAST", ("op", 2)],
        16: [("op", 3)],
    }

    def flush_av(prev):
        ppc, pqi, pets = prev
        av = av_block(ppc, pqi, pets)
        epilogue(ppc, pqi, av)

    prev = None  # (pc, qi, ets) awaiting AV+epilogue
    for t in range(17):
        cur = None
        if t < 16:
            pc, qi = t // 4, t % 4
            cur = (pc, qi, scores_block(pc, qi))
        for tok in pre_av.get(t, []):
            run_token(tok)
        if prev is not None:
            flush_av(prev)
            prev = None
        prev = cur
        for tok in post.get(t, []):
            if tok == "AV_LAST":
                flush_av(prev)
                prev = None
            elif tok == "WV":
                nc.sync.dma_start(out=wv_sb, in_=wvt_r)
            elif tok == "WO":
                nc.sync.dma_start(out=wo_sb, in_=wot_r)
            else:
                run_token(tok)


def _build():
    global _cached_nc
    if _cached_nc is not None:
        return _cached_nc
    nc = bacc.Bacc("TRN2", target_bir_lowering=False, debug=False)
    io = {
        "qt": nc.dram_tensor("qt", [D, S], BF16, kind="ExternalInput"),
        "kt": nc.dram_tensor("kt", [D, S], BF16, kind="ExternalInput"),
        "vt": nc.dram_tensor("vt", [D, S], BF16, kind="ExternalInput"),
        "wqt": nc.dram_tensor("wqt", [D, DOUT], BF16, kind="ExternalInput"),
        "wkt": nc.dram_tensor("wkt", [D, DOUT], BF16, kind="ExternalInput"),
        "wvt": nc.dram_tensor("wvt", [D, DOUT], BF16, kind="ExternalInput"),
        "wot": nc.dram_tensor("wot", [DOUT, D], BF16, kind="ExternalInput"),
        "bq": nc.dram_tensor("bq", [DOUT], F32, kind="ExternalInput"),
        "bk": nc.dram_tensor("bk", [DOUT], F32, kind="ExternalInput"),
        "bv": nc.dram_tensor("bv", [DOUT], F32, kind="ExternalInput"),
        "out": nc.dram_tensor("out", [S, D], F32, kind="ExternalOutput"),
    }
    with tile.TileContext(nc) as tc:
        with ExitStack() as ctx:
            _emit(ctx, tc, io)
    nc.compile()
    _cached_nc = nc
    return nc


def make_in_maps(Q, K, V, Wq, bq, Wk, bk, Wv, bv, Wo):
    bf = lambda a: np.ascontiguousarray(np.asarray(a, np.float32)).astype(
        ml_dtypes.bfloat16
    )
    f = lambda a: np.ascontiguousarray(a, dtype=np.float32)
    in_maps = []
    for c in range(N_CORES):
        b = c // 2
        lo = (c % 2) * DOUT
        sl = slice(lo, lo + DOUT)
        in_maps.append(
            {
                "qt": bf(np.asarray(Q, np.float32)[b].T),
                "kt": bf(np.asarray(K, np.float32)[b].T),
                "vt": bf(np.asarray(V, np.float32)[b].T),
                "wqt": bf(np.asarray(Wq, np.float32)[sl, :].T),
                "wkt": bf(np.asarray(Wk, np.float32)[sl, :].T),
                "wvt": bf(np.asarray(Wv, np.float32)[sl, :].T),
                "wot": bf(np.asarray(Wo, np.float32)[:, sl].T),
                "bq": f(bq[sl]),
                "bk": f(bk[sl]),
                "bv": f(bv[sl]),
            }
        )
    return in_maps


def gather_output(results, bo):
    out = np.empty((B, S, D), dtype=np.float32)
    bo = np.asarray(bo, dtype=np.float32)
    for b in range(B):
        out[b] = results[2 * b]["out"] + results[2 * b + 1]["out"] + bo
    return out


def _numpy_fallback(Q, K, V, mask, Wq, bq, Wk, bk, Wv, bv, Wo, bo):
    """Exact reference math in numpy (only used if mask isn't all-ones)."""
    H, dk = 16, 64
    out = np.empty((B, S, D), dtype=np.float32)
    for b in range(B):
        q = (Q[b] @ Wq.T + bq).reshape(S, H, dk).transpose(1, 0, 2)
        k = (K[b] @ Wk.T + bk).reshape(S, H, dk).transpose(1, 0, 2)
        v = (V[b] @ Wv.T + bv).reshape(S, H, dk).transpose(1, 0, 2)
        o = np.empty((H, S, dk), dtype=np.float32)
        for h in range(H):
            s = (q[h] @ k[h].T) / np.sqrt(np.float32(dk))
            s = np.where(mask[b] == 0, np.float32(-1.0e9), s)
            s = s - s.max(axis=-1, keepdims=True)
            e = np.exp(s)
            a = e / e.sum(axis=-1, keepdims=True)
            o[h] = a @ v[h]
        out[b] = o.transpose(1, 0, 2).reshape(S, H * dk) @ Wo.T + bo
    return out


def kernel(Q, K, V, mask, Wq, bq, Wk, bk, Wv, bv, Wo, bo):
    Q = np.asarray(Q, dtype=np.float32)
    K = np.asarray(K, dtype=np.float32)
    V = np.asarray(V, dtype=np.float32)
    Wq = np.asarray(Wq, dtype=np.float32)
    Wk = np.asarray(Wk, dtype=np.float32)
    Wv = np.asarray(Wv, dtype=np.float32)
    Wo = np.asarray(Wo, dtype=np.float32)
    bq = np.asarray(bq, dtype=np.float32)
    bk = np.asarray(bk, dtype=np.float32)
    bv = np.asarray(bv, dtype=np.float32)
    bo = np.asarray(bo, dtype=np.float32)
    mask_np = np.asarray(mask)

    if not np.all(mask_np != 0):
        return _numpy_fallback(Q, K, V, mask_np, Wq, bq, Wk, bk, Wv, bv, Wo, bo)

    nc = _build()
    in_maps = make_in_maps(Q, K, V, Wq, bq, Wk, bk, Wv, bv, Wo)
    res = run_bass_kernel_spmd(nc, in_maps, list(range(N_CORES))).results
    return gather_output(res, bo)
